# revision 38
# baseline (speedup 1.0000x reference)
"""Causal self-attention (12 heads, T=1024, C=768, prefix P=4) on 8 TRN2 cores.

Sharding: data-parallel over batch B=8 -> one batch element per NeuronCore.
No collectives. Weights are replicated to every core.

Design (fp16 matmul inputs, fp32 PSUM accumulation; ~165us HW vs 207us
for the f32r/per-head baseline):
  - bias folding: k-bias dropped (softmax shift-invariance; prefix k gets
    b_k subtracted host-side), v-bias dropped (b_p' = b_v @ w_p + b_p;
    prefix v gets b_v subtracted host-side). Only q-bias applied on device
    (Scalar engine activation with per-partition bias AP).
  - scores: the two heads of a pair run as CONCURRENT row-tiled matmuls
    (K=64 strips) into the two banks of a [128, 2, 512] PSUM tile; one exp
    drains both banks into a per-(pair, window) e supertile. Diagonal-band
    tiles compute/exp only cols j0:512; a [128, 2, 128] triangular mask
    multiply fixes the diagonal blocks.
  - prefix scores: 4 col-tiled concurrent matmuls (M=4, strips 32j,
    j=2s+c) into one PSUM bank; one exp for all 4.
  - AV: y^T accumulation per (head, window) over kv chunks; prefix (K=4
    at strip 32j) appended last. Denominator = v column 64 (ones).
  - norm: py -> sbuf copy, reciprocal_approx_fast over rows 0-65 (custom
    DVE op needs base partition 0; only row 64 is read), K=1 f32r matmul
    broadcasts 1/denom to [64, W], DVE multiply writes yT fp16.
  - out proj: yT fp16 x w_p fp16, bias add fused with PSUM drain.
  - emission: score slots (exp-gated through a 2-deep PSUM ring) are
    woven between dense PE work (vproj / qkproj / AV) so the in-order PE
    queue never waits on the Scalar engine.
"""

import numpy as np
from contextlib import ExitStack

import concourse.bass as bass
import concourse.mybir as mybir
import concourse.tile as tile
from concourse import bacc
from concourse.bass_utils import run_bass_kernel_spmd

F32 = mybir.dt.float32
F32R = mybir.dt.float32r
F16 = mybir.dt.float16
N_CORES = 8
T, C, H, D, PFX = 1024, 768, 12, 64, 4
NPAIR = H // 2          # 6 head pairs
KC = C // 128           # 6 contraction chunks
W = 512                 # T window
NW = T // W             # 2 windows
TCH = T // 128          # 8 T chunks
EXP = mybir.ActivationFunctionType.Exp
SCALE = 1.0 / np.sqrt(D)


def _build():
    nc = bacc.Bacc("TRN2", target_bir_lowering=False, debug=False,
                   num_devices=N_CORES)
    xT_d = nc.declare_dram_parameter("xT", [C, T], F16, isOutput=False)
    wqk_d = nc.declare_dram_parameter("w_qk", [C, 2 * C], F16, isOutput=False)
    wv_d = nc.declare_dram_parameter("w_v", [C, C], F16, isOutput=False)
    wp_d = nc.declare_dram_parameter("w_p", [C, C], F16, isOutput=False)
    bq_d = nc.declare_dram_parameter("b_q", [128, NPAIR], F32, isOutput=False)
    bp_d = nc.declare_dram_parameter("bp_bc", [128, C], F32, isOutput=False)
    kTc_d = nc.declare_dram_parameter("kTc", [C, PFX], F16, isOutput=False)
    vc2_d = nc.declare_dram_parameter("vc2", [128, NPAIR, 128], F16,
                                      isOutput=False)
    tri2_d = nc.declare_dram_parameter("tri2", [128, 2, 128], F16,
                                       isOutput=False)
    out_d = nc.declare_dram_parameter("out", [T, C], F16, isOutput=True)

    with tile.TileContext(nc) as tc, ExitStack() as ctx:
        pers = ctx.enter_context(tc.tile_pool(name="pers", bufs=1))
        wqkp = ctx.enter_context(tc.tile_pool(name="wqkp", bufs=18))
        qkhp = ctx.enter_context(tc.tile_pool(name="qkhp", bufs=2))
        ep0 = ctx.enter_context(tc.tile_pool(name="ep0", bufs=2))
        ep1 = ctx.enter_context(tc.tile_pool(name="ep1", bufs=2))
        etpp = ctx.enter_context(tc.tile_pool(name="etpp", bufs=2))
        rwp = ctx.enter_context(tc.tile_pool(name="rwp", bufs=2))
        op = ctx.enter_context(tc.tile_pool(name="op", bufs=2))
        psq = ctx.enter_context(tc.tile_pool(name="psq", bufs=2, space="PSUM"))
        ps = ctx.enter_context(tc.tile_pool(name="ps", bufs=2, space="PSUM"))
        pyp = ctx.enter_context(tc.tile_pool(name="pyp", bufs=2, space="PSUM"))

        # ---- persistent loads: wave 1 = exactly what vblock(0) touches,
        # wave 2 = vblocks 1-3, wave 3 = the rest. wp/bp are emitted at the
        # end of the startup block (only needed by outproj).
        xtb = [pers.tile([128, T], F16, tag=f"xtb{k}", name=f"xtb{k}")
               for k in range(KC)]
        wv = [pers.tile([128, C], F16, tag=f"wv{k}", name=f"wv{k}")
              for k in range(KC)]
        for k in range(KC):
            nc.sync.dma_start(xtb[k][:, 0:W],
                              xT_d[128 * k:128 * k + 128, 0:W])
            nc.sync.dma_start(wv[k][:, 0:384],
                              wv_d[128 * k:128 * k + 128, 0:384])

        def load_wave2():
            for k in range(KC):
                nc.sync.dma_start(wv[k][:, 384:C],
                                  wv_d[128 * k:128 * k + 128, 384:C])
        bq = pers.tile([128, NPAIR], F32, tag="bq")
        nc.sync.dma_start(bq[:], bq_d[:])
        tri2 = pers.tile([128, 2, 128], F16, tag="tri2")
        nc.sync.dma_start(tri2[:], tri2_d[:])
        vc2 = pers.tile([128, NPAIR, 128], F16, tag="vc2")
        ones66 = pers.tile([66, 64], F16, tag="ones66")
        nc.vector.memset(ones66[64:66, :], 1.0)

        def load_wave3():
            for k in range(KC):
                nc.sync.dma_start(xtb[k][:, W:768],
                                  xT_d[128 * k:128 * k + 128, W:768])
            for k in range(KC):
                nc.sync.dma_start(xtb[k][:, 768:T],
                                  xT_d[128 * k:128 * k + 128, 768:T])
            nc.sync.dma_start(vc2[:], vc2_d[:])

        yT = [pers.tile([128, T], F16, tag=f"yT{p}", name=f"yT{p}")
              for p in range(NPAIR)]

        vt = []
        for mt in range(TCH):
            v_ = pers.tile([128, H, 128], F16, tag=f"v{mt}", name=f"v{mt}")
            nc.vector.memset(v_[:, :, 64:65], 1.0)
            nc.vector.memset(v_[:, :, 65:128], 0.0)
            vt.append(v_)

        wp = [pers.tile([128, C], F16, tag=f"wp{k}", name=f"wp{k}")
              for k in range(KC)]
        bp = pers.tile([128, C], F32, tag="bp")

        def load_wp():
            for k in range(KC):
                nc.sync.dma_start(wp[k][:], wp_d[128 * k:128 * k + 128, :])
            nc.sync.dma_start(bp[:], bp_d[:])

        qk_tiles = {}
        esups = {}
        etps = {}
        pys = {}

        def vhalf(mt, half):
            pv = ps.tile([128, W], F32, tag="ps", name=f"pv{mt}_{half}")
            for k in range(KC):
                nc.tensor.matmul(pv[:, 0:384],
                                 xtb[k][:, 128 * mt:128 * mt + 128],
                                 wv[k][:, 384 * half:384 * half + 384],
                                 start=(k == 0), stop=(k == KC - 1))
            nc.vector.tensor_copy(
                vt[mt][:, 6 * half:6 * half + 6, 0:64],
                pv[:, 0:384].rearrange("a (h d) -> a h d", d=64))

        def qk_dma(p):
            wq = []
            for k in range(KC):
                t_ = wqkp.tile([128, 2, 128], F16, tag="wqk",
                               name=f"wq{p}_{k}")
                src = wqk_d[128 * k:128 * k + 128, :].rearrange(
                    "a (s b) -> a s b", s=2)[:, :, 128 * p:128 * p + 128]
                nc.sync.dma_start(t_[:], src)
                wq.append(t_)
            qT = qkhp.tile([128, T], F16, tag="qT", name=f"qT{p}")
            kh = qkhp.tile([128, T + PFX], F16, tag="kh", name=f"kh{p}")
            qk_tiles[p] = (qT, kh, wq)
            nc.sync.dma_start(kh[:, T:T + PFX],
                              kTc_d[128 * p:128 * p + 128, :])

        def qkw(p, w):
            """q + k projection chains for window w of pair p."""
            if w == 0 and p not in qk_tiles:
                qk_dma(p)
            qT, kh, wq = qk_tiles[p]
            pq = ps.tile([128, W], F32, tag="ps", name=f"pq{p}_{w}")
            for k in range(KC):
                nc.tensor.matmul(pq[:], wq[k][:, 0, :],
                                 xtb[k][:, W * w:W * w + W],
                                 start=(k == 0), stop=(k == KC - 1))
            nc.scalar.add(qT[:, W * w:W * w + W], pq[:], bq[:, p:p + 1])
            pk = ps.tile([128, W], F32, tag="ps", name=f"pk{p}_{w}")
            for k in range(KC):
                nc.tensor.matmul(pk[:], wq[k][:, 1, :],
                                 xtb[k][:, W * w:W * w + W],
                                 start=(k == 0), stop=(k == KC - 1))
            nc.vector.tensor_copy(kh[:, W * w:W * w + W], pk[:])

        def scores_open(p):
            for c in range(NW):
                nr = 4 * c + 4
                esups[(p, c)] = (ep0 if c == 0 else ep1).tile(
                    [128, 2, nr * W], F16, tag=f"es{c}", name=f"es{p}_{c}")

        def slot(p, c, r):
            qT, kh, _ = qk_tiles[p]
            esup = esups[(p, c)]
            j0 = max(0, 128 * r - W * c)
            pt = psq.tile([128, 2, W], F32, tag="psq", name=f"pss{p}_{c}_{r}")
            for s in range(2):
                nc.tensor.matmul(
                    pt[:, s, j0:W],
                    kh[64 * s:64 * s + 64, 128 * r:128 * r + 128],
                    qT[64 * s:64 * s + 64, W * c + j0:W * (c + 1)],
                    start=True, stop=True)
            nc.scalar.activation(esup[:, :, W * r + j0:W * (r + 1)],
                                 pt[:, :, j0:W], EXP, scale=float(SCALE))

        def tri(p, c):
            esup = esups[(p, c)]
            for i in range(4):
                r = 4 * c + i
                j0 = 128 * r - W * c
                nc.vector.tensor_mul(esup[:, :, W * r + j0:W * r + j0 + 128],
                                     esup[:, :, W * r + j0:W * r + j0 + 128],
                                     tri2[:])

        def prefix_scores(p):
            qT, kh, _ = qk_tiles[p]
            pp = pyp.tile([128, W], F32, tag="py", name=f"pp{p}")
            for s in range(2):
                for c in range(NW):
                    j = 2 * s + c
                    nc.tensor.matmul(pp[32 * j:32 * j + PFX, :],
                                     kh[64 * s:64 * s + 64, T:T + PFX],
                                     qT[64 * s:64 * s + 64, W * c:W * (c + 1)],
                                     start=True, stop=True,
                                     tile_position=(64 * s, 32 * j))
            etp = etpp.tile([128, W], F16, tag="etp", name=f"etp{p}")
            etps[p] = etp
            nc.scalar.activation(etp[:], pp[:], EXP, scale=float(SCALE))

        def av(p, s, c):
            h = 2 * p + s
            py = pyp.tile([128, W], F32, tag="py", name=f"py{p}_{s}_{c}")
            pys[(p, s, c)] = py
            esup = esups[(p, c)]
            for r in range(4 * c + 4):
                tstart = max(0, 128 * r - W * c)
                nc.tensor.matmul(py[:, tstart:W],
                                 vt[r][:, h, :],
                                 esup[:, s, W * r + tstart:W * (r + 1)],
                                 start=(r == 0), stop=False)
            j = 2 * s + c
            nc.tensor.matmul(py[:, :], vc2[32 * j:32 * j + PFX, p, :],
                             etps[p][32 * j:32 * j + PFX, :],
                             start=False, stop=True,
                             tile_position=(32 * j, 0))

        norm_state = {}

        def norms_pre(p, c):
            sbs, rr = {}, {}
            for s in range(2):
                py = pys[(p, s, c)]
                sb = rwp.tile([66, W], F32, tag="sb", name=f"sb{p}_{s}_{c}")
                sbs[s] = sb
                nc.vector.tensor_copy(sb[:], py[0:66, :])
            for s in range(2):
                rrow = rwp.tile([66, W], F32, tag="rwf", name=f"rwf{p}_{s}_{c}")
                rr16 = rwp.tile([66, W], F16, tag="rw", name=f"rw{p}_{s}_{c}")
                # custom-DVE op only routes correctly from base partition 0;
                # rows 0-63 produce garbage that is never read (only row 64,
                # the denominator, feeds the broadcast matmul).
                nc.vector.reciprocal_approx_fast(rrow[0:66, :],
                                                 sbs[s][0:66, :])
                with nc.allow_low_precision(reason="fp16 1/denom, |err|<6e-4"):
                    nc.vector.tensor_copy(rr16[64:65, :], rrow[64:65, :])
                rr[s] = rr16
            norm_state[(p, c)] = (sbs, rr)

        def norms_post(p, c):
            sbs, rr = norm_state[(p, c)]
            pbs = {}
            for s in range(2):
                pb = ps.tile([128, W], F32, tag="ps", name=f"pb{p}_{s}_{c}")
                pbs[s] = pb
                nc.tensor.matmul(pb[0:64, :], ones66[64:65, :],
                                 rr[s][64:65, :], start=True, stop=True)
            for s in range(2):
                nc.vector.tensor_mul(
                    yT[p][64 * s:64 * s + 64, W * c:W * c + W],
                    sbs[s][0:64, :], pbs[s][0:64, :])

        def norms(p, c):
            norms_pre(p, c)
            norms_post(p, c)

        po_tiles = {}

        def outproj_a(mt):
            po = psq.tile([128, 2, W], F32, tag="psq", name=f"po{mt}")
            po_tiles[mt] = po
            for half in range(2):
                for kp in range(NPAIR - 1):
                    nc.tensor.matmul(
                        po[:, half, 0:384],
                        yT[kp][:, 128 * mt:128 * mt + 128],
                        wp[kp][:, 384 * half:384 * half + 384],
                        start=(kp == 0), stop=False)

        def outproj_b(mt):
            po = po_tiles[mt]
            kp = NPAIR - 1
            for half in range(2):
                nc.tensor.matmul(
                    po[:, half, 0:384],
                    yT[kp][:, 128 * mt:128 * mt + 128],
                    wp[kp][:, 384 * half:384 * half + 384],
                    start=False, stop=True)
            osb = op.tile([128, C], F16, tag="osb", name=f"osb{mt}")
            nc.vector.tensor_add(
                osb[:].rearrange("a (s x) -> a s x", s=2),
                po[:, :, 0:384], bp[:].rearrange("a (s x) -> a s x", s=2))
            nc.sync.dma_start(out_d[128 * mt:128 * mt + 128, :], osb[:])

        def outproj(mts):
            for mt in mts:
                outproj_a(mt)
                outproj_b(mt)

        # ---- emission schedule: weave exp-gated score slots into dense
        # PE work so the in-order PE queue never waits on the Scalar engine.
        qk_dma(0)        # DMA-queue position: right after wave 1
        load_wave2()
        load_wave3()
        vhalf(0, 0)
        vhalf(1, 0)
        qkw(0, 0)
        vhalf(2, 0)
        vhalf(3, 0)
        scores_open(0)
        slot(0, 0, 0)
        slot(0, 0, 1)
        vhalf(4, 0)
        slot(0, 0, 2)
        slot(0, 0, 3)
        tri(0, 0)
        qkw(0, 1)
        vhalf(5, 0)
        slot(0, 1, 4)
        slot(0, 1, 5)
        vhalf(6, 0)
        vhalf(7, 0)
        slot(0, 1, 6)
        slot(0, 1, 7)
        vhalf(0, 1)
        vhalf(1, 1)
        slot(0, 1, 0)
        slot(0, 1, 1)
        vhalf(2, 1)
        vhalf(3, 1)
        slot(0, 1, 2)
        slot(0, 1, 3)
        tri(0, 1)
        vhalf(4, 1)
        prefix_scores(0)
        qk_dma(1)
        vhalf(5, 1)
        vhalf(6, 1)
        vhalf(7, 1)

        for p in range(NPAIR):
            last = p == NPAIR - 1
            if not last:
                qkw(p + 1, 0)
                scores_open(p + 1)
            av(p, 0, 0)
            if p == 1:
                load_wp()
            if not last:
                slot(p + 1, 0, 0)
                slot(p + 1, 0, 1)
            av(p, 1, 0)
            if not last:
                slot(p + 1, 0, 2)
                slot(p + 1, 0, 3)
                tri(p + 1, 0)
            if not last:
                norms_pre(p, 0)
                qkw(p + 1, 1)
                slot(p + 1, 1, 4)
                slot(p + 1, 1, 5)
                norms_post(p, 0)
            else:
                norms_pre(p, 0)
                outproj_a(0)
                outproj_a(1)
                norms_post(p, 0)
                outproj_b(0)
                outproj_b(1)
                outproj(range(2, 4))
            av(p, 0, 1)
            if p + 2 < NPAIR:
                qk_dma(p + 2)
            if not last:
                slot(p + 1, 1, 6)
                slot(p + 1, 1, 7)
                slot(p + 1, 1, 0)
            av(p, 1, 1)
            if not last:
                slot(p + 1, 1, 1)
                slot(p + 1, 1, 2)
                norms_pre(p, 1)
                slot(p + 1, 1, 3)
                tri(p + 1, 1)
                prefix_scores(p + 1)
                norms_post(p, 1)
            if last:
                norms_pre(p, 1)
                outproj_a(4)
                outproj_a(5)
                norms_post(p, 1)
                outproj_b(4)
                outproj_b(5)
        outproj(range(6, TCH))

    nc.finalize()
    return nc


def _prep_inputs(x, kv_cvec, w_attn, b_attn, w_proj, b_proj):
    x = np.asarray(x, np.float32)
    kv_cvec = np.asarray(kv_cvec, np.float32)
    w_attn = np.asarray(w_attn, np.float32)
    b_attn = np.asarray(b_attn, np.float32)
    w_proj = np.asarray(w_proj, np.float32)
    b_proj = np.asarray(b_proj, np.float32)
    bq, bk, bv = b_attn[:C], b_attn[C:2 * C], b_attn[2 * C:]

    tri = (np.arange(128)[:, None] <= np.arange(128)[None, :])
    tri2 = np.broadcast_to(tri[:, None, :], (128, 2, 128))

    shared = {
        "w_qk": np.ascontiguousarray(w_attn[:, :2 * C]).astype(np.float16),
        "w_v": np.ascontiguousarray(w_attn[:, 2 * C:]).astype(np.float16),
        "w_p": np.ascontiguousarray(w_proj).astype(np.float16),
        "b_q": np.ascontiguousarray(bq.reshape(NPAIR, 128).T),
        "bp_bc": np.ascontiguousarray(
            np.broadcast_to(b_proj + bv @ w_proj, (128, C))),
        "tri2": np.ascontiguousarray(tri2).astype(np.float16),
    }
    in_maps = []
    for b in range(N_CORES):
        kc = kv_cvec[b][:, :C] - bk          # [PFX, C]
        vc = kv_cvec[b][:, C:] - bv          # [PFX, C]
        vc2 = np.zeros((128, NPAIR, 128), np.float32)
        for p in range(NPAIR):
            for s in range(2):
                h = 2 * p + s
                for c in range(2):
                    j = 2 * s + c
                    vc2[32 * j:32 * j + PFX, p, 0:64] = \
                        vc[:, 64 * h:64 * h + 64]
                    vc2[32 * j:32 * j + PFX, p, 64] = 1.0
        m = dict(shared)
        m["xT"] = np.ascontiguousarray(x[b].T).astype(np.float16)
        m["kTc"] = np.ascontiguousarray(kc.T).astype(np.float16)
        m["vc2"] = vc2.astype(np.float16)
        in_maps.append(m)
    return in_maps


_NC_CACHE = {}


def run_hw(trace=False, tmpdir=None, **inputs):
    """Build+compile+run on 8 NeuronCores; returns (out [8,1024,768], results)."""
    if "nc" not in _NC_CACHE:
        _NC_CACHE["nc"] = _build()
    nc = _NC_CACHE["nc"]
    in_maps = _prep_inputs(**inputs)
    res = run_bass_kernel_spmd(nc, in_maps, list(range(N_CORES)), trace=trace,
                               tmpdir=tmpdir)
    out = np.stack([res.results[b]["out"] for b in range(N_CORES)]
                   ).astype(np.float32)
    return out, res


def kernel(**inputs):
    out, _ = run_hw(trace=False, **inputs)
    return out
